# revision 1
# baseline (speedup 1.0000x reference)
"""MultiHeadGraphAttention TRN2 kernel.

Data-parallel over (batch, query-half): core c handles batch c//2, query rows
(c%2)*1024 .. +1024.  Attention rows are independent, so there are no
collectives.  All matmuls run in bf16 (fp32 PSUM accumulation); softmax and
LayerNorm run in fp32.

Layout trick: scores are computed TRANSPOSED (S^T[m, n], key positions on
partitions), so softmax needs no on-chip transposes anywhere:
  - exp on ScalarE (PSUM -> SBUF bf16, 2 score tiles per op), mask multiply
    on VectorE
  - denominator comes free from an appended ones-column on V (row 64 of the
    AV matmul output)
  - O^T [hd, n] feeds the output projection directly as lhsT, and Y lands in
    straight [n, d] layout for residual + LayerNorm.

Perf notes (from NTFF traces): if the PE sees a ~3.4us idle window the HAM
clock gate drops it from 2.4 to 1.2 GHz and it can only recover via ~3.4us of
UNINTERRUPTED matmul activity — which a softmax-paced stream never supplies.
So the V projection is interleaved into the first attention head: the PE
always has dependency-free projection matmuls to chew on while the first
exp/mask round-trips fill the pipeline.  Loops run n-chunk-outer so the
output projection + LayerNorm of chunk 0 overlap the attention of chunk 1.
"""

import os
import sys

import numpy as np

try:
    import concourse  # noqa: F401
except ImportError:  # harness runs from a bare dir; the repo is a fixed path
    sys.path.insert(0, "/opt/trn_rl_repo")

import ml_dtypes

B, N, M, D, H, HD = 4, 2048, 2048, 512, 8, 64
NS = 1024          # query rows per core
NCORES = 8
LN_EPS = 1e-5
BF16 = ml_dtypes.bfloat16

_CACHE = {}

# experiment knobs (read once at build)
K_TRECIP = int(os.environ.get("K_TRECIP", "1"))  # PE-transpose reciprocal


def _build():
    import concourse.bass as bass  # noqa: F401
    import concourse.tile as tile
    from concourse import bacc, mybir
    from concourse.masks import make_identity

    f32 = mybir.dt.float32
    bf16 = mybir.dt.bfloat16
    Exp = mybir.ActivationFunctionType.Exp
    Sqrt = mybir.ActivationFunctionType.Sqrt
    sub = mybir.AluOpType.subtract
    mult = mybir.AluOpType.mult

    nc = bacc.Bacc(None, target_bir_lowering=False, debug=False)

    xqT_d = nc.dram_tensor("xqT", [D, NS], bf16, kind="ExternalInput")
    xkT_d = nc.dram_tensor("xkT", [D, M], bf16, kind="ExternalInput")
    xvT_d = nc.dram_tensor("xvT", [D, M], bf16, kind="ExternalInput")
    maskT_d = nc.dram_tensor("maskT", [M, NS], bf16, kind="ExternalInput")
    qres_d = nc.dram_tensor("qres", [NS, D], f32, kind="ExternalInput")
    wqT_d = nc.dram_tensor("wqT", [D, D], bf16, kind="ExternalInput")
    wkT_d = nc.dram_tensor("wkT", [D, D], bf16, kind="ExternalInput")
    wvT_d = nc.dram_tensor("wvT", [D, D], bf16, kind="ExternalInput")
    woT_d = nc.dram_tensor("woT", [D, D], bf16, kind="ExternalInput")
    gamma_d = nc.dram_tensor("gamma", [1, D], f32, kind="ExternalInput")
    beta_d = nc.dram_tensor("beta", [1, D], f32, kind="ExternalInput")
    out_d = nc.dram_tensor("out", [NS, D], f32, kind="ExternalOutput")

    KC = D // 128      # 4 contraction chunks of 128
    NT = NS // 128     # 8 query-row tiles
    NCH = NS // 512    # 2 query-column chunks for matmul free dim
    MT = M // 128      # 16 key-position tiles
    MCH = M // 512     # 4 key chunks of 512
    MG = MT // 2       # 8 score groups (2 key tiles per exp/mask op)

    with tile.TileContext(nc) as tc:
        with (
            tc.tile_pool(name="big", bufs=1) as big,
            tc.tile_pool(name="wpool", bufs=1) as wpool,
            tc.tile_pool(name="ppool", bufs=3) as ppool,
            tc.tile_pool(name="opool", bufs=3) as opool,
            tc.tile_pool(name="ypool", bufs=3) as ypool,
            tc.tile_pool(name="small", bufs=4) as small,
            tc.tile_pool(name="ps_mm", bufs=2, space="PSUM") as ps_mm,
            tc.tile_pool(name="ps_s", bufs=2, space="PSUM") as ps_s,
            tc.tile_pool(name="ps_o", bufs=2, space="PSUM") as ps_o,
        ):
            # ---- resident SBUF tensors -----------------------------------
            xqT = big.tile([128, KC, NS], bf16, tag="xqT")
            xkT = big.tile([128, KC, M], bf16, tag="xkT")
            xvT = big.tile([128, KC, M], bf16, tag="xvT")
            maskT = big.tile([128, MT, NS], bf16, tag="maskT")
            qT = big.tile([128, KC, NS], bf16, tag="qT")
            kT = big.tile([128, KC, M], bf16, tag="kT")
            vS = big.tile([128, MT, H * (HD + 1)], bf16, tag="vS")
            oT = big.tile([128, KC, NS], bf16, tag="oT")
            wq = wpool.tile([128, KC, D], bf16, tag="wq")
            wk = wpool.tile([128, KC, D], bf16, tag="wk")
            wv = wpool.tile([128, KC, D], bf16, tag="wv")
            wo = wpool.tile([128, KC, D], bf16, tag="wo")
            gamma_b = wpool.tile([128, D], f32, tag="gamma_b")
            beta_b = wpool.tile([128, D], f32, tag="beta_b")
            gamma_1 = wpool.tile([1, D], f32, tag="gamma_1")
            beta_1 = wpool.tile([1, D], f32, tag="beta_1")
            eps_t = wpool.tile([128, 1], f32, tag="eps")
            if K_TRECIP:
                ident = wpool.tile([128, 128], f32, tag="ident")
                make_identity(nc, ident)

            # ---- input DMAs ----------------------------------------------
            nc.sync.dma_start(out=xqT, in_=xqT_d[:].rearrange("(c p) n -> p c n", p=128))
            nc.sync.dma_start(out=xkT, in_=xkT_d[:].rearrange("(c p) n -> p c n", p=128))
            nc.sync.dma_start(out=xvT, in_=xvT_d[:].rearrange("(c p) n -> p c n", p=128))
            for j in range(MT):
                nc.sync.dma_start(
                    out=maskT[:, j, :],
                    in_=maskT_d[:].rearrange("(j p) n -> p j n", p=128)[:, j, :],
                )
            nc.sync.dma_start(out=wq, in_=wqT_d[:].rearrange("(c p) o -> p c o", p=128))
            nc.sync.dma_start(out=wk, in_=wkT_d[:].rearrange("(c p) o -> p c o", p=128))
            nc.sync.dma_start(out=wv, in_=wvT_d[:].rearrange("(c p) o -> p c o", p=128))
            nc.sync.dma_start(out=wo, in_=woT_d[:].rearrange("(c p) o -> p c o", p=128))
            nc.sync.dma_start(out=gamma_1, in_=gamma_d[:])
            nc.sync.dma_start(out=beta_1, in_=beta_d[:])
            nc.gpsimd.partition_broadcast(gamma_b, gamma_1, channels=128)
            nc.gpsimd.partition_broadcast(beta_b, beta_1, channels=128)
            nc.vector.memset(eps_t, LN_EPS)
            # ones column per head in the augmented V (gives the softmax
            # denominator as row 64 of the AV matmul output)
            nc.vector.memset(
                vS[:].rearrange("p j (h x) -> p j h x", x=HD + 1)[:, :, :, HD : HD + 1],
                1.0,
            )

            # ---- projection emitters -------------------------------------
            def q_proj(t, ncc):
                ps = ps_mm.tile([128, 512], f32, tag="mm")
                for kc in range(KC):
                    nc.tensor.matmul(
                        ps,
                        lhsT=wq[:, kc, t * 128 : (t + 1) * 128],
                        rhs=xqT[:, kc, ncc * 512 : (ncc + 1) * 512],
                        start=(kc == 0),
                        stop=(kc == KC - 1),
                    )
                nc.scalar.copy(qT[:, t, ncc * 512 : (ncc + 1) * 512], ps)

            def k_proj(t, mc):
                ps = ps_mm.tile([128, 512], f32, tag="mm")
                for kc in range(KC):
                    nc.tensor.matmul(
                        ps,
                        lhsT=wk[:, kc, t * 128 : (t + 1) * 128],
                        rhs=xkT[:, kc, mc * 512 : (mc + 1) * 512],
                        start=(kc == 0),
                        stop=(kc == KC - 1),
                    )
                nc.scalar.copy(kT[:, t, mc * 512 : (mc + 1) * 512], ps)

            def v_proj(j):
                # V[m, o] straight, scattered into per-head 65-wide slots
                ps = ps_mm.tile([128, 512], f32, tag="mm")
                for kc in range(KC):
                    nc.tensor.matmul(
                        ps,
                        lhsT=xvT[:, kc, j * 128 : (j + 1) * 128],
                        rhs=wv[:, kc, :],
                        start=(kc == 0),
                        stop=(kc == KC - 1),
                    )
                nc.vector.tensor_copy(
                    out=vS[:, j, :].rearrange("p (h x) -> p h x", x=HD + 1)[:, :, 0:HD],
                    in_=ps[:].rearrange("p (h x) -> p h x", x=HD),
                )

            # ---- attention head emitter ----------------------------------
            def attend(h, ncc, filler=None):
                po = (h % 2) * 64
                t = h // 2
                nsl = slice(ncc * 512, (ncc + 1) * 512)
                po_t = ps_o.tile([HD + 1, 512], f32, tag="po")
                for g in range(MG):           # 2 key tiles per group
                    ps2 = ps_s.tile([128, 1024], f32, tag="s")
                    for u in range(2):
                        j = 2 * g + u
                        nc.tensor.matmul(
                            ps2[:, u * 512 : (u + 1) * 512],
                            lhsT=kT[po : po + 64, t, j * 128 : (j + 1) * 128],
                            rhs=qT[po : po + 64, t, nsl],
                            start=True,
                            stop=True,
                        )
                    if filler is not None:
                        filler(g)
                    pt = ppool.tile([128, 1024], bf16, tag="pt")
                    nc.scalar.activation(pt, ps2, Exp)
                    nc.vector.tensor_mul(
                        pt.rearrange("p (u n) -> p u n", u=2),
                        pt.rearrange("p (u n) -> p u n", u=2),
                        maskT[:, 2 * g : 2 * g + 2, nsl],
                    )
                    for u in range(2):
                        j = 2 * g + u
                        nc.tensor.matmul(
                            po_t,
                            lhsT=vS[:, j, h * (HD + 1) : (h + 1) * (HD + 1)],
                            rhs=pt[:, u * 512 : (u + 1) * 512],
                            start=(j == 0),
                            stop=(j == MT - 1),
                        )
                # normalize: O^T = O_u^T * (1/d) broadcast over partitions
                recip = small.tile([1, 512], f32, tag="recip")
                if K_TRECIP:
                    d_sb = small.tile([1, 512], f32, tag="d_sb")
                    nc.vector.tensor_copy(out=d_sb, in_=po_t[64:65, :])
                    scr = ps_mm.tile([128, 512], f32, tag="mm")
                    dT = scr[:, 0:4]
                    rrow = scr[0:1, 0:512]
                    for c in range(KC):
                        nc.tensor.transpose(
                            dT[:, c : c + 1], d_sb[:, c * 128 : (c + 1) * 128], ident[0:1, 0:1]
                        )
                    rT = small.tile([128, 4], f32, tag="rT")
                    nc.vector.reciprocal(rT, dT)
                    for c in range(KC):
                        nc.tensor.transpose(
                            rrow[:, c * 128 : (c + 1) * 128], rT[:, c : c + 1], ident
                        )
                    nc.vector.tensor_copy(out=recip, in_=rrow)
                else:
                    nc.vector.reciprocal(recip, po_t[64:65, :])
                recip_b = opool.tile([64, 512], f32, tag="recip_b")
                nc.gpsimd.partition_broadcast(recip_b, recip, channels=64)
                nc.vector.tensor_mul(oT[po : po + 64, t, nsl], po_t[0:64, :], recip_b)

            # ---- output projection + residual + LayerNorm ----------------
            qres_r = qres_d[:].rearrange("(t p) d -> p t d", p=128)
            out_r = out_d[:].rearrange("(t p) d -> p t d", p=128)

            def out_tile(nt):
                ps = ps_mm.tile([128, 512], f32, tag="mm")
                for a in range(KC):
                    nc.tensor.matmul(
                        ps,
                        lhsT=oT[:, a, nt * 128 : (nt + 1) * 128],
                        rhs=wo[:, a, :],
                        start=(a == 0),
                        stop=(a == KC - 1),
                    )
                qres_t = ypool.tile([128, D], f32, tag="qres")
                nc.sync.dma_start(out=qres_t, in_=qres_r[:, nt, :])
                x_t = ypool.tile([128, D], f32, tag="x")
                nc.vector.tensor_add(x_t, ps, qres_t)
                stats = small.tile([128, 6], f32, tag="stats")
                nc.vector.bn_stats(out=stats, in_=x_t)
                mv = small.tile([128, 2], f32, tag="mv")
                nc.vector.bn_aggr(out=mv, in_=stats)
                rstd = small.tile([128, 1], f32, tag="rstd")
                nc.scalar.activation(rstd, mv[:, 1:2], Sqrt, bias=eps_t)
                nc.vector.reciprocal(rstd, rstd)
                xn = ypool.tile([128, D], f32, tag="xn")
                nc.vector.tensor_scalar(
                    out=xn, in0=x_t, scalar1=mv[:, 0:1], scalar2=rstd, op0=sub, op1=mult
                )
                y_t = ypool.tile([128, D], f32, tag="y")
                nc.gpsimd.tensor_mul(y_t, xn, gamma_b)
                nc.vector.tensor_add(y_t, y_t, beta_b)
                nc.sync.dma_start(out=out_r[:, nt, :], in_=y_t)

            # ---- emission schedule ---------------------------------------
            # Q and K projections up front; V projection interleaved into the
            # first attention head so the PE never idles while the first
            # exp/mask round-trips prime the softmax pipeline.
            for t in range(KC):
                for ncc in range(NCH):
                    q_proj(t, ncc)
            for t in range(KC):
                for mc in range(MCH):
                    k_proj(t, mc)
            for j in range(6):
                v_proj(j)

            def v_filler(g):
                for j in (6 + 2 * g, 7 + 2 * g):
                    if j < MT:
                        v_proj(j)

            for h in range(H):
                attend(h, 0, filler=v_filler if h == 0 else None)
            for h in range(H):
                attend(h, 1)
                if h < 4:
                    out_tile(h)      # chunk-0 output overlaps chunk-1 attention
            for nt in range(4, 8):
                out_tile(nt)

    nc.compile()
    return nc


def kernel(**inputs):
    from concourse.bass_utils import run_bass_kernel_spmd

    if "nc" not in _CACHE:
        _CACHE["nc"] = _build()
    nc = _CACHE["nc"]

    query = np.asarray(inputs["query"], dtype=np.float32)
    key = np.asarray(inputs["key"], dtype=np.float32)
    value = np.asarray(inputs["value"], dtype=np.float32)
    mask = np.asarray(inputs["mask"])
    WQ = np.asarray(inputs["WQ"], dtype=np.float32)
    WK = np.asarray(inputs["WK"], dtype=np.float32)
    WV = np.asarray(inputs["WV"], dtype=np.float32)
    WO = np.asarray(inputs["WO"], dtype=np.float32)
    bO = np.asarray(inputs["bO"], dtype=np.float32)
    gamma = np.asarray(inputs["gamma"], dtype=np.float32)
    beta = np.asarray(inputs["beta"], dtype=np.float32)

    scale = np.float32(1.0 / np.sqrt(HD))
    wqT = np.ascontiguousarray(WQ.T * scale).astype(BF16)
    wkT = np.ascontiguousarray(WK.T).astype(BF16)
    wvT = np.ascontiguousarray(WV.T).astype(BF16)
    woT = np.ascontiguousarray(WO.T).astype(BF16)
    gamma_in = gamma.reshape(1, D)
    beta_in = beta.reshape(1, D)
    mask_bin = (mask != 0)

    in_maps = []
    for c in range(NCORES):
        b, n0 = c // 2, (c % 2) * NS
        in_maps.append({
            "xqT": np.ascontiguousarray(query[b, n0 : n0 + NS, :].T).astype(BF16),
            "xkT": np.ascontiguousarray(key[b].T).astype(BF16),
            "xvT": np.ascontiguousarray(value[b].T).astype(BF16),
            "maskT": np.ascontiguousarray(mask_bin[b, n0 : n0 + NS, :].T).astype(BF16),
            "qres": np.ascontiguousarray(query[b, n0 : n0 + NS, :] + bO[None, :]),
            "wqT": wqT, "wkT": wkT, "wvT": wvT, "woT": woT,
            "gamma": gamma_in, "beta": beta_in,
        })

    trace = bool(int(os.environ.get("BASS_KERNEL_TRACE", "0")))
    res = run_bass_kernel_spmd(nc, in_maps, core_ids=list(range(NCORES)), trace=trace)
    _CACHE["last_results"] = res

    out = np.empty((B, N, D), dtype=np.float32)
    for c in range(NCORES):
        b, n0 = c // 2, (c % 2) * NS
        out[b, n0 : n0 + NS, :] = res.results[c]["out"]
    return out



# revision 16
# speedup vs baseline: 1.1260x; 1.1260x over previous
"""MultiHeadGraphAttention TRN2 kernel (v2 — ACT-paced pipeline).

Data-parallel over (batch, query-half): core c handles batch c//2, query rows
(c%2)*1024 .. +1024.  No collectives.  Matmuls in bf16 (fp32 PSUM), softmax
and LayerNorm in fp32.

The softmax exp is the hard engine floor: 16.8M score elements must pass
through ScalarE's ACTIVATE(Exp) at 1 elem/cycle/lane @1.2GHz = ~143us/core.
Everything is organized so ACT runs exp back-to-back and every other engine
hides underneath:

  - scores are computed TRANSPOSED (S^T[m, n]) in [128,1024] PSUM tiles
    (2 key tiles x 512 query cols), exp'd PSUM->SBUF bf16 by ACT, masked by
    DVE (multiply by 0/1 mask AFTER exp), then AV-matmul'd with an appended
    ones-column on V giving the softmax denominator for free.
  - score matmuls for an even/odd head pair are emitted adjacently with
    lhsT/rhs at base_partition 0/64, so they land on PE row-tiles T0/T8
    (64x128 mode) and stream CONCURRENTLY -> scores cost half the cycles.
  - the unit stream runs one unit of score-lookahead ahead of exp; AV and
    the Q/K/V/O projection matmuls trail as PE filler in ACT's slack.
  - LayerNorm: stats on DVE inline, but the Sqrt for all 8 row tiles is
    batched at the very end so ACT never thrashes activation table sets.
"""

import os
import sys

import numpy as np

try:
    import concourse  # noqa: F401
except ImportError:  # harness runs from a bare dir; the repo is a fixed path
    sys.path.insert(0, "/opt/trn_rl_repo")

import ml_dtypes

B, N, M, D, H, HD = 4, 2048, 2048, 512, 8, 64
NS = 1024          # query rows per core
NCORES = 8
LN_EPS = 1e-5
BF16 = ml_dtypes.bfloat16

_CACHE = {}

KC = D // 128      # 4 contraction chunks of 128
NT = NS // 128     # 8 query-row tiles
NCH = NS // 512    # 2 query-column chunks
MT = M // 128      # 16 key-position tiles
MCH = M // 512     # 4 key chunks of 512
NU = MT // 2       # 8 units per chunk (2 key tiles each)


def _build():
    import concourse.bass as bass  # noqa: F401
    import concourse.tile as tile
    from concourse import bacc, mybir
    from concourse.masks import make_identity

    f32 = mybir.dt.float32
    bf16 = mybir.dt.bfloat16
    Exp = mybir.ActivationFunctionType.Exp
    Sqrt = mybir.ActivationFunctionType.Sqrt
    sub = mybir.AluOpType.subtract
    mult = mybir.AluOpType.mult

    nc = bacc.Bacc(None, target_bir_lowering=False, debug=False)

    xqT_d = nc.dram_tensor("xqT", [D, NS], bf16, kind="ExternalInput")
    xkT_d = nc.dram_tensor("xkT", [D, M], bf16, kind="ExternalInput")
    xvT_d = nc.dram_tensor("xvT", [D, M], bf16, kind="ExternalInput")
    maskT_d = nc.dram_tensor("maskT", [M, NS], bf16, kind="ExternalInput")
    qres_d = nc.dram_tensor("qres", [NS, D], f32, kind="ExternalInput")
    wqT_d = nc.dram_tensor("wqT", [D, D], bf16, kind="ExternalInput")
    wkT_d = nc.dram_tensor("wkT", [D, D], bf16, kind="ExternalInput")
    wvT_d = nc.dram_tensor("wvT", [D, D], bf16, kind="ExternalInput")
    woT_d = nc.dram_tensor("woT", [D, D], bf16, kind="ExternalInput")
    gamma_d = nc.dram_tensor("gamma", [1, D], f32, kind="ExternalInput")
    beta_d = nc.dram_tensor("beta", [1, D], f32, kind="ExternalInput")
    out_d = nc.dram_tensor("out", [NS, D], f32, kind="ExternalOutput")

    with tile.TileContext(nc) as tc:
        with (
            tc.tile_pool(name="big", bufs=1) as big,
            tc.tile_pool(name="wpool", bufs=1) as wpool,
            tc.tile_pool(name="ptp", bufs=3) as ptp,
            tc.tile_pool(name="ypool", bufs=2) as ypool,
            tc.tile_pool(name="opool", bufs=2) as opool,
            tc.tile_pool(name="small", bufs=4) as small,
            tc.tile_pool(name="nsc", bufs=1) as nsc,
            tc.tile_pool(name="xvp", bufs=4) as xvp,
            tc.tile_pool(name="ps_s", bufs=2, space="PSUM") as ps_s,
            tc.tile_pool(name="ps_mm", bufs=2, space="PSUM") as ps_mm,
            tc.tile_pool(name="ps_o", bufs=2, space="PSUM") as ps_o,
        ):
            # ---- resident SBUF tensors -----------------------------------
            xqT = big.tile([128, KC, NS], bf16, tag="xqT")
            xkT = big.tile([128, KC, M], bf16, tag="xkT")
            maskT = big.tile([128, MT, NS], bf16, tag="maskT")
            qT = big.tile([128, KC, NS], bf16, tag="qT")
            kT = big.tile([128, KC, M], bf16, tag="kT")
            vS = big.tile([128, MT, H * (HD + 1)], bf16, tag="vS")
            oT = big.tile([128, KC, NS], bf16, tag="oT")
            wq = wpool.tile([128, KC, D], bf16, tag="wq")
            wk = wpool.tile([128, KC, D], bf16, tag="wk")
            wv = wpool.tile([128, KC, D], bf16, tag="wv")
            wo = wpool.tile([128, KC, D], bf16, tag="wo")
            gamma_b = wpool.tile([128, D], f32, tag="gamma_b")
            gamma_bb = wpool.tile([128, D], bf16, tag="gamma_bb")
            beta_b = wpool.tile([128, D], f32, tag="beta_b")
            gamma_1 = wpool.tile([1, D], f32, tag="gamma_1")
            beta_1 = wpool.tile([1, D], f32, tag="beta_1")
            eps_t = wpool.tile([128, 1], f32, tag="eps")
            ident = wpool.tile([128, 128], f32, tag="ident")
            # LN staging (persist until the batched tail)
            vars8 = wpool.tile([128, NT], f32, tag="vars8")
            g_all = wpool.tile([128, NT, D], bf16, tag="g_all")
            make_identity(nc, ident)

            # ---- input DMAs ----------------------------------------------
            nc.sync.dma_start(out=xqT, in_=xqT_d[:].rearrange("(c p) n -> p c n", p=128))
            nc.sync.dma_start(out=xkT, in_=xkT_d[:].rearrange("(c p) n -> p c n", p=128))
            for j in range(MT):
                nc.sync.dma_start(
                    out=maskT[:, j, :],
                    in_=maskT_d[:].rearrange("(j p) n -> p j n", p=128)[:, j, :],
                )
            nc.sync.dma_start(out=wq, in_=wqT_d[:].rearrange("(c p) o -> p c o", p=128))
            nc.sync.dma_start(out=wk, in_=wkT_d[:].rearrange("(c p) o -> p c o", p=128))
            nc.sync.dma_start(out=wv, in_=wvT_d[:].rearrange("(c p) o -> p c o", p=128))
            nc.sync.dma_start(out=wo, in_=woT_d[:].rearrange("(c p) o -> p c o", p=128))
            nc.sync.dma_start(out=gamma_1, in_=gamma_d[:])
            nc.sync.dma_start(out=beta_1, in_=beta_d[:])
            nc.gpsimd.partition_broadcast(gamma_b, gamma_1, channels=128)
            nc.gpsimd.partition_broadcast(beta_b, beta_1, channels=128)
            nc.vector.tensor_copy(out=gamma_bb, in_=gamma_b)
            nc.vector.memset(eps_t, LN_EPS)
            nc.vector.memset(
                vS[:].rearrange("p j (h x) -> p j h x", x=HD + 1)[:, :, :, HD : HD + 1],
                1.0,
            )

            # ---- projection emitters (PE filler work) --------------------
            def q_proj(t, ncc):
                ps = ps_mm.tile([128, 512], f32, tag="mm")
                for kc in range(KC):
                    nc.tensor.matmul(
                        ps,
                        lhsT=wq[:, kc, t * 128 : (t + 1) * 128],
                        rhs=xqT[:, kc, ncc * 512 : (ncc + 1) * 512],
                        start=(kc == 0),
                        stop=(kc == KC - 1),
                    )
                nc.vector.tensor_copy(
                    out=qT[:, t, ncc * 512 : (ncc + 1) * 512], in_=ps
                )

            def k_proj(t, mc):
                ps = ps_mm.tile([128, 512], f32, tag="mm")
                for kc in range(KC):
                    nc.tensor.matmul(
                        ps,
                        lhsT=wk[:, kc, t * 128 : (t + 1) * 128],
                        rhs=xkT[:, kc, mc * 512 : (mc + 1) * 512],
                        start=(kc == 0),
                        stop=(kc == KC - 1),
                    )
                nc.vector.tensor_copy(
                    out=kT[:, t, mc * 512 : (mc + 1) * 512], in_=ps
                )

            xvT_r = xvT_d[:].rearrange("(c p) (j m) -> p c j m", p=128, m=128)

            def v_proj(j):
                xv_t = xvp.tile([128, KC, 128], bf16, tag="xv")
                nc.sync.dma_start(out=xv_t, in_=xvT_r[:, :, j, :])
                ps = ps_mm.tile([128, 512], f32, tag="mm")
                for kc in range(KC):
                    nc.tensor.matmul(
                        ps,
                        lhsT=xv_t[:, kc, :],
                        rhs=wv[:, kc, :],
                        start=(kc == 0),
                        stop=(kc == KC - 1),
                    )
                nc.vector.tensor_copy(
                    out=vS[:, j, :].rearrange("p (h x) -> p h x", x=HD + 1)[:, :, 0:HD],
                    in_=ps[:].rearrange("p (h x) -> p h x", x=HD),
                )

            # ---- out-projection stage 1 (per row tile, inline) -----------
            qres_r = qres_d[:].rearrange("(t p) d -> p t d", p=128)
            out_r = out_d[:].rearrange("(t p) d -> p t d", p=128)

            def out_tile_s1(nt):
                ps = ps_mm.tile([128, D], f32, tag="mm")
                for a in range(KC):
                    nc.tensor.matmul(
                        ps,
                        lhsT=oT[:, a, nt * 128 : (nt + 1) * 128],
                        rhs=wo[:, a, :],
                        start=(a == 0),
                        stop=(a == KC - 1),
                    )
                x_t = ypool.tile([128, D], f32, tag="qres")
                nc.sync.dma_start(out=x_t, in_=qres_r[:, nt, :])
                nc.vector.tensor_add(x_t, ps, x_t)
                stats = small.tile([128, 6], f32, tag="stats")
                nc.vector.bn_stats(out=stats, in_=x_t)
                mv = small.tile([128, 2], f32, tag="mv")
                nc.vector.bn_aggr(out=mv, in_=stats)
                nc.vector.tensor_copy(out=vars8[:, nt : nt + 1], in_=mv[:, 1:2])
                nc.vector.tensor_scalar(
                    out=g_all[:, nt, :], in0=x_t, scalar1=mv[:, 0:1],
                    scalar2=None, op0=sub,
                )
                nc.gpsimd.tensor_mul(g_all[:, nt, :], g_all[:, nt, :], gamma_bb)

            def ln_tail():
                rstd = wpool.tile([128, NT], f32, tag="rstd8")
                nc.scalar.activation(rstd, vars8, Sqrt, bias=eps_t)
                nc.vector.reciprocal(rstd, rstd)
                for nt in range(NT):
                    y_t = ypool.tile([128, D], f32, tag="y")
                    nc.vector.tensor_scalar(
                        out=y_t,
                        in0=g_all[:, nt, :],
                        scalar1=rstd[:, nt : nt + 1],
                        scalar2=None,
                        op0=mult,
                    )
                    nc.vector.tensor_add(y_t, y_t, beta_b)
                    nc.sync.dma_start(out=out_r[:, nt, :], in_=y_t)

            # ---- softmax denominator reciprocal (PE transpose trick) -----
            def normalize(po_t, h, ncc):
                po = (h % 2) * 64
                t = h // 2
                nsl = slice(ncc * 512, (ncc + 1) * 512)
                d_sb = nsc.tile([1, 512], f32, tag="d_sb")
                nc.vector.tensor_copy(out=d_sb, in_=po_t[64:65, :])
                scr = ps_mm.tile([128, 512], f32, tag="mm")
                dT = scr[:, 0:4]
                rrow = scr[0:1, 0:512]
                for c in range(KC):
                    nc.tensor.transpose(
                        dT[:, c : c + 1], d_sb[:, c * 128 : (c + 1) * 128], ident[0:1, 0:1]
                    )
                rT = small.tile([128, 4], f32, tag="rT")
                nc.vector.reciprocal(rT, dT)
                for c in range(KC):
                    nc.tensor.transpose(
                        rrow[:, c * 128 : (c + 1) * 128], rT[:, c : c + 1], ident
                    )
                recip = nsc.tile([1, 512], f32, tag="recip")
                nc.vector.tensor_copy(out=recip, in_=rrow)
                recip_b = opool.tile([64, 512], f32, tag="recip_b")
                nc.gpsimd.partition_broadcast(recip_b, recip, channels=64)
                nc.vector.tensor_mul(oT[po : po + 64, t, nsl], po_t[0:64, :], recip_b)

            # ---- the ACT-paced unit stream -------------------------------
            # chunk c = (hp, ncc), ncc-major: heads 2hp, 2hp+1; 512 q-cols.
            chunks = [(hp, ncc) for ncc in range(NCH) for hp in range(KC)]

            # deadline-driven filler: todo[ci] MUST be fully emitted before
            # chunk ci's first s_unit (Tile serializes on trace order, so a
            # score matmul emitted before its projection reads garbage).
            todo = {ci: [] for ci in range(1 + KC * NCH)}
            for t in range(1, KC):
                for mc in range(MCH):
                    todo[t].append(lambda t=t, mc=mc: k_proj(t, mc))
                todo[t].append(lambda t=t: q_proj(t, 0))
            for t in range(KC):
                todo[KC + t].append(lambda t=t: q_proj(t, 1))

            def s_unit(hp, ncc, u):
                """Score matmuls for unit u of chunk (hp,ncc): row-tile pair."""
                nsl = slice(ncc * 512, (ncc + 1) * 512)
                tiles = []
                for par in range(2):       # 0: even head (T0), 1: odd (T8)
                    po = par * 64
                    ps2 = ps_s.tile([128, 1024], f32, tag="s")
                    for w in range(2):
                        j = 2 * u + w
                        nc.tensor.matmul(
                            ps2[:, w * 512 : (w + 1) * 512],
                            lhsT=kT[po : po + 64, hp, j * 128 : (j + 1) * 128],
                            rhs=qT[po : po + 64, hp, nsl],
                            start=True,
                            stop=True,
                        )
                    tiles.append(ps2)
                return tiles

            # prelude: just enough projections for chunk 0's first scores
            k_proj(0, 0)
            q_proj(0, 0)
            for j in range(4):
                v_proj(j)

            pend = {}                      # (ci, u) -> [psA, psB]
            po_tiles = {}                  # ci -> [po_e, po_o]
            pend[(0, 0)] = s_unit(0, 0, 0)

            for ci, (hp, ncc) in enumerate(chunks):
                nsl = slice(ncc * 512, (ncc + 1) * 512)
                ptE = ptp.tile([128, MT, 512], bf16, tag="pt")
                ptO = ptp.tile([128, MT, 512], bf16, tag="pt")
                pts = [ptE, ptO]
                po_e = ps_o.tile([HD + 1, 512], f32, tag="po")
                po_o = ps_o.tile([HD + 1, 512], f32, tag="po")
                po_tiles[ci] = [po_e, po_o]

                nxt = todo.get(ci + 1, [])
                npop = 0
                for u in range(NU):
                    tiles = pend.pop((ci, u))
                    # exp + mask for both parities of this unit
                    for par in range(2):
                        pt_sl = pts[par][:, 2 * u : 2 * u + 2, :]
                        nc.scalar.activation(pt_sl, tiles[par], Exp)
                        nc.vector.tensor_mul(
                            pt_sl, pt_sl, maskT[:, 2 * u : 2 * u + 2, nsl]
                        )
                    # next chunk's projections must all precede its scores;
                    # drain the remainder before the u=NU-1 lookahead.
                    quota = (
                        len(nxt)
                        if u == NU - 2
                        else (u + 1) * len(nxt) // (NU - 1)
                    )
                    while npop < quota:
                        nxt[npop]()
                        npop += 1
                    # lookahead scores: next unit (crossing chunk boundary)
                    if u < NU - 1:
                        pend[(ci, u + 1)] = s_unit(hp, ncc, u + 1)
                    elif ci + 1 < len(chunks):
                        nhp, nncc = chunks[ci + 1]
                        pend[(ci + 1, 0)] = s_unit(nhp, nncc, 0)
                    # AV for this unit, both parities
                    for par in range(2):
                        h = 2 * hp + par
                        for w in range(2):
                            j = 2 * u + w
                            nc.tensor.matmul(
                                po_tiles[ci][par],
                                lhsT=vS[:, j, h * (HD + 1) : (h + 1) * (HD + 1)],
                                rhs=pts[par][:, j, :],
                                start=(j == 0),
                                stop=(j == MT - 1),
                            )
                    # chunk-0 JIT work: rest of K(t=0) + V tiles ahead of AV
                    if ci == 0:
                        if u in (0, 2, 4):
                            k_proj(0, u // 2 + 1)
                        if u < NU - 2:     # v_proj JIT: 2 units lookahead
                            v_proj(2 * u + 4)
                            v_proj(2 * u + 5)

                # retire the chunk's heads
                normalize(po_e, 2 * hp, ncc)
                normalize(po_o, 2 * hp + 1, ncc)
                # out-projection stage 1 for completed n-blocks
                if 4 <= ci < 8:
                    out_tile_s1(ci - 4)    # nt 0..3 during chunks 4..7

            for nt in range(4, NT):
                out_tile_s1(nt)
            ln_tail()

    nc.compile()
    return nc


def kernel(**inputs):
    from concourse.bass_utils import run_bass_kernel_spmd

    if "nc" not in _CACHE:
        _CACHE["nc"] = _build()
    nc = _CACHE["nc"]

    query = np.asarray(inputs["query"], dtype=np.float32)
    key = np.asarray(inputs["key"], dtype=np.float32)
    value = np.asarray(inputs["value"], dtype=np.float32)
    mask = np.asarray(inputs["mask"])
    WQ = np.asarray(inputs["WQ"], dtype=np.float32)
    WK = np.asarray(inputs["WK"], dtype=np.float32)
    WV = np.asarray(inputs["WV"], dtype=np.float32)
    WO = np.asarray(inputs["WO"], dtype=np.float32)
    bO = np.asarray(inputs["bO"], dtype=np.float32)
    gamma = np.asarray(inputs["gamma"], dtype=np.float32)
    beta = np.asarray(inputs["beta"], dtype=np.float32)

    scale = np.float32(1.0 / np.sqrt(HD))
    wqT = np.ascontiguousarray(WQ.T * scale).astype(BF16)
    wkT = np.ascontiguousarray(WK.T).astype(BF16)
    wvT = np.ascontiguousarray(WV.T).astype(BF16)
    woT = np.ascontiguousarray(WO.T).astype(BF16)
    gamma_in = gamma.reshape(1, D)
    beta_in = beta.reshape(1, D)
    mask_bin = (mask != 0)

    in_maps = []
    for c in range(NCORES):
        b, n0 = c // 2, (c % 2) * NS
        in_maps.append({
            "xqT": np.ascontiguousarray(query[b, n0 : n0 + NS, :].T).astype(BF16),
            "xkT": np.ascontiguousarray(key[b].T).astype(BF16),
            "xvT": np.ascontiguousarray(value[b].T).astype(BF16),
            "maskT": np.ascontiguousarray(mask_bin[b, n0 : n0 + NS, :].T).astype(BF16),
            "qres": np.ascontiguousarray(query[b, n0 : n0 + NS, :] + bO[None, :]),
            "wqT": wqT, "wkT": wkT, "wvT": wvT, "woT": woT,
            "gamma": gamma_in, "beta": beta_in,
        })

    trace = bool(int(os.environ.get("BASS_KERNEL_TRACE", "0")))
    res = run_bass_kernel_spmd(nc, in_maps, core_ids=list(range(NCORES)), trace=trace)
    _CACHE["last_results"] = res

    out = np.empty((B, N, D), dtype=np.float32)
    for c in range(NCORES):
        b, n0 = c // 2, (c % 2) * NS
        out[b, n0 : n0 + NS, :] = res.results[c]["out"]
    return out
